# revision 60
# baseline (speedup 1.0000x reference)
"""Trainium2 Bass kernel for nn_CPDist.

Math: with a = exp(h_last @ W.T + b).reshape(B, H, V, R), the reference
computes p_tilde[b,i,j] = sum_r a[b,0,i,r]*a[b,1,j,r], then
  p_eval[b]     = p_tilde[b, p0, p1]
  norm_const[b] = sum_ij p_tilde[b,i,j]
Both factorize over the rank dim, so the (B,V,V) slab is never needed:
  norm_const[b] = sum_r (sum_i a[b,0,i,r]) * (sum_j a[b,1,j,r])
  p_eval[b]     = sum_r a[b,0,p0,r] * a[b,1,p1,r]
The dominant cost is the (B=8, D=1024) x (D, V*R*H=131072) matmul + exp —
HBM-bound on streaming the 512 MB weight matrix (fp8: 16.8 MB/core).

Sharding: vocab dim V split across 8 cores (512 vocab rows each, for both
horizon slots). Each core streams its (1024, 16384) transposed weight slab
through the PE array against a stationary h^T and applies exp on the scalar
engine, whose accum_out gives the per-(h,r) vocab-sum partials.

Pipeline-tail structure: the last RAWN chunks stream as a piece list (TAIL)
and skip the activation accumulator — their exp values ship raw (fp16) in
the output and are summed on host, which shortens the serial post-stream
chain (dma-sem 900ns + matmul + exp) that gates the output DMA.  The final
piece streams as SUBLAST sub-DMAs feeding one psum tile so its matmuls
overlap the very last transfer, and the output leaves in two DMAs (bulk
early, the last pieces' columns late).  The p_eval gather path (256 rows of
W) is sharded across cores (32 columns each, fp8, riding inside a mid-
stream weight DMA) and also shipped raw; the host picks the diagonal
entries.

Per-core W slab column order is (h, r, v) so each 512-column matmul chunk
is exactly one (h, r) pair over all 512 local vocab entries.  pack2 (the
DoubleRow-interleaved fp8 h^T) rides in the same DMA as the first weight
chunk.
"""

import os

import numpy as np

import concourse.bacc as bacc
import concourse.bass as bass
import concourse.mybir as mybir
import concourse.tile as tile

B, T, D = 8, 128, 1024
V, R, H = 4096, 16, 2
NCORES = 8
VSH = V // NCORES            # vocab rows per core (512)
CHUNK = VSH                  # columns per matmul chunk
NCHUNK = H * R               # 32 chunks of 512 columns = 16384 per core
KT = D // 128                # 8 contraction tiles
NG = B * H * R               # 256 gathered columns for p_eval (global)
NGC = NG // NCORES           # 32 gathered columns per core
NHRC = NCHUNK // NCORES      # 4 (h,r) pairs per core on the gather path

F32 = mybir.dt.float32
F32R = mybir.dt.float32r
BF16 = mybir.dt.bfloat16
GDT = mybir.dt.float16

_MM_NAME = os.environ.get("CPDIST_MM_DTYPE", "float8e4")
MM_DTYPE = getattr(mybir.dt, _MM_NAME)
# fp8 operands are pre-scaled into e4m3's sweet spot; the activation's scale
# argument undoes S*S on the logits before exp.
MM_SCALE = 1024.0 if MM_DTYPE == mybir.dt.float8e4 else 1.0
DR = os.environ.get("CPDIST_DR", "1" if MM_DTYPE == mybir.dt.float8e4 else "0") == "1"

# number of trailing chunks whose exp values ship raw (host sums them);
# their weight columns stream as the TAIL piece list below
RAWN = int(os.environ.get("CPDIST_RAWN", "6"))
# main-stream group plan for chunks 1..(31-RAWN) (chunk 0 rides with pack2)
_PLAN = os.environ.get("CPDIST_PLAN", "1,2,4,4,4,4,4,2")
PLAN = [int(x) for x in _PLAN.split(",") if x]
assert sum(PLAN) == NCHUNK - 1 - RAWN, (PLAN, RAWN)
# piece widths for the last RAWN chunks (streamed last, raw-shipped)
_TAIL = os.environ.get("CPDIST_TAIL", "512,512,512,512,512,256,256")
TAIL = [int(x) for x in _TAIL.split(",") if x]
assert sum(TAIL) == RAWN * CHUNK, (TAIL, RAWN)
SIDE_AT = int(os.environ.get("CPDIST_SIDE_AT", "3"))
# number of trailing TAIL pieces whose raw columns ship in a second, tiny
# output DMA (the bulk ships as soon as its writers finish)
OSPLIT = int(os.environ.get("CPDIST_OSPLIT", "2"))
# sub-DMA count for the final TAIL piece (its matmuls overlap the last
# transfer; sub-width must be a multiple of 64 to stay at full DMA rate)
SUBLAST = int(os.environ.get("CPDIST_SUBLAST", "4"))
# how many trailing TAIL pieces stream as sub-DMAs (subs of max(64, w//4))
# so each piece's matmuls overlap its own transfer; their tiles come from a
# dedicated pool with enough buffers that every sub pre-issues early
SUBN = int(os.environ.get("CPDIST_SUBN", "1"))

WBUFS = int(os.environ.get("CPDIST_WBUFS", "6"))
PSBUFS = int(os.environ.get("CPDIST_PSBUFS", "7"))

# output layout: sg_out (f32) holds the accumulator-summed s columns;
# sg_out16 (fp16) holds the raw gather-exp matrix followed by the raw
# tail-chunk e values (host sums those)
NS = NCHUNK - RAWN                  # accumulator-summed s columns
SG_RAW = NGC                        # raw cols start after eg in sg_out16
SG16_COLS = NGC + RAWN * CHUNK


def _piece_subs(pi, w):
    """Sub-DMA width for TAIL piece pi; == w means a single DMA."""
    if pi >= len(TAIL) - SUBN:
        sub = max(64, w // 4)
        if w % sub == 0 and sub % 64 == 0:
            return sub
    return w


_N_WHOLE = sum(1 for i, w in enumerate(TAIL) if _piece_subs(i, w) == w)
_N_SUBS = sum(w // _piece_subs(i, w)
              for i, w in enumerate(TAIL) if _piece_subs(i, w) != w)

# pack2 column layout (mm dtype): DoubleRow-interleaved h^T only (k-pairs
# padded to 16-col stride so the dual-fp8 LDWEIGHTS pair stride is 16 bytes);
# the plain per-k h^T views index into the same region
P2_DR = 0
P2_COLS = P2_DR + (KT // 2) * 32

# pack1 (f32) single row: onesf | biasg (per-core gathered bias, x S^2)
P1_ONES = 0
P1_BIASG = P1_ONES + B
P1_COLS = P1_BIASG + NGC

# wg (mm dtype): per-core gather matrix, k-tiled, x MM_SCALE — rides inside
# the SIDE_AT weight-group DMA (the gather matmul reuses the fp8 h^T from c0)
WG_COLS = KT * NGC

# wtc fp8 tensor: [pack2 | chunk 0 | chunks 1.. with wg after the SIDE_AT
# group | tail pieces]
W0_OFF = P2_COLS
WTC_COLS = W0_OFF + NCHUNK * KT * CHUNK + WG_COLS

_cached = {}
_fast = {}
_last_results = None


def _round_fp32r(x):
    u = x.view(np.uint32)
    u = (u + np.uint32(0x7FF) + ((u >> np.uint32(12)) & np.uint32(1))) & np.uint32(
        0xFFFFF000
    )
    return u.view(np.float32)


def _to_mm(x, scale=1.0):
    x = np.ascontiguousarray(x, dtype=np.float32)
    if scale != 1.0:
        x = x * np.float32(scale)
    if MM_DTYPE == mybir.dt.float32r:
        return _round_fp32r(x)
    return x.astype(mybir.dt.np(MM_DTYPE))


def _build_nc(mm_dtype, nloop=1, use_bias=True):
    nc = bacc.Bacc("TRN2", target_bir_lowering=False)
    pack1 = nc.dram_tensor("pack1", (1, P1_COLS), F32, kind="ExternalInput")
    wtc = nc.dram_tensor("wtc", (128, WTC_COLS), mm_dtype, kind="ExternalInput")
    bias_m = nc.dram_tensor("bias_m", (1, NCHUNK * CHUNK + B), BF16, kind="ExternalInput")
    sg_out = nc.dram_tensor("sg_out", (B, NS), F32, kind="ExternalOutput")
    sg_out16 = nc.dram_tensor("sg_out16", (B, SG16_COLS), GDT, kind="ExternalOutput")

    with tile.TileContext(nc) as tc:
        with (
            tc.tile_pool(name="consts", bufs=1) as consts,
            tc.tile_pool(name="wpool", bufs=WBUFS) as wpool,
            tc.tile_pool(name="lpool", bufs=max(1, _N_WHOLE)) as lpool,
            tc.tile_pool(name="spool", bufs=max(1, _N_SUBS)) as spool,
            tc.tile_pool(name="pspool", bufs=PSBUFS, space="PSUM") as pspool,
            tc.tile_pool(name="psg_pool", bufs=1, space="PSUM") as psg_pool,
            tc.tile_pool(name="epool", bufs=3) as epool,
            tc.tile_pool(name="opool", bufs=1) as opool,
        ):
            # ---- head DMAs: first weight chunk (with pack2) before the
            # small consts, so the weight stream owns the DMA engines from
            # the first possible cycle with no ordering bubbles.
            c0 = consts.tile([128, W0_OFF + KT * CHUNK], mm_dtype)
            nc.sync.dma_start(out=c0[:], in_=wtc[:, 0:W0_OFF + KT * CHUNK])
            src_pos = [W0_OFF + KT * CHUNK]

            def issue_group(nch, extra=0):
                ncols = KT * CHUNK * nch + extra
                w_tile = wpool.tile([128, ncols], mm_dtype,
                                    padded_shape=[128, KT * CHUNK * 4 + WG_COLS],
                                    name=f"w_tile_{src_pos[0]}", tag="w_tile")
                nc.sync.dma_start(
                    out=w_tile[:],
                    in_=wtc[:, src_pos[0]:src_pos[0] + ncols],
                )
                src_pos[0] += ncols
                return w_tile

            pre = [issue_group(PLAN[0])]

            bias_sb = consts.tile([1, NCHUNK * CHUNK + B], BF16)
            nc.sync.dma_start(out=bias_sb[:], in_=bias_m[:])
            ones_r = bias_sb[0:1, NCHUNK * CHUNK:NCHUNK * CHUNK + B]

            pre.append(issue_group(PLAN[1]))

            p1_sb = consts.tile([1, P1_COLS], F32)
            wg_view = [None]

            def wg_k(k):
                return wg_view[0][:, k * NGC:(k + 1) * NGC]

            onesf_sb = p1_sb[0:1, P1_ONES:P1_ONES + B]
            biasg_sb = p1_sb[0:1, P1_BIASG:P1_BIASG + NGC]

            def ht_k(k):
                off = P2_DR + (k // 2) * 32 + (k % 2) * 16
                return c0[:, off:off + B]

            sg_sb = opool.tile([B, NS], F32)
            sg16_sb = opool.tile([B, SG16_COLS], GDT)

            def emit_side_path():
                # p_eval gathered factors (this core's 32 of 256 columns);
                # raw exp matrix lands in sg16 for host-side diagonal pick
                nc.sync.dma_start(out=p1_sb[:], in_=pack1[:])
                psg = psg_pool.tile([B, NGC], F32)
                for k in range(KT):
                    nc.tensor.matmul(
                        psg[:], lhsT=ht_k(k), rhs=wg_k(k),
                        start=(k == 0), stop=False,
                    )
                nc.tensor.matmul(
                    psg[:], lhsT=onesf_sb, rhs=biasg_sb, start=False, stop=True
                )
                nc.scalar.activation(
                    sg16_sb[:, 0:NGC], psg[:],
                    mybir.ActivationFunctionType.Exp,
                    scale=1.0 / (MM_SCALE * MM_SCALE),
                )

            def chunk_mms(half, w_ap, off, ch, width):
                """bias matmul + DoubleRow (or plain) K-tile matmuls into a
                psum slice of `width` columns for chunk `ch`, whose weight
                columns start at element offset `off` of w_ap."""
                if use_bias:
                    nc.tensor.matmul(
                        half,
                        lhsT=ones_r,
                        rhs=bias_sb[:, ch * CHUNK:ch * CHUNK + width],
                        start=True,
                        stop=False,
                    )
                if DR:
                    for k2 in range(KT // 2):
                        nc.tensor.matmul(
                            half,
                            lhsT=c0[:, P2_DR + k2 * 32:P2_DR + (k2 + 1) * 32]
                                .rearrange("p (i m) -> p i m", i=2)[:, :, 0:B],
                            rhs=w_ap[:, off + 2 * k2 * width:
                                     off + (2 * k2 + 2) * width]
                                .rearrange("p (i n) -> p i n", i=2),
                            start=(not use_bias and k2 == 0),
                            stop=(k2 == KT // 2 - 1),
                            perf_mode=mybir.MatmulPerfMode.DoubleRow,
                        )
                else:
                    for k in range(KT):
                        nc.tensor.matmul(
                            half,
                            lhsT=ht_k(k),
                            rhs=w_ap[:, off + k * width:off + (k + 1) * width],
                            start=(not use_bias and k == 0),
                            stop=(k == KT - 1),
                        )

            def chunk_compute(ch, w_ap, jbase, raw_off=None):
                ps = pspool.tile([B, CHUNK], F32, tag="ps")
                chunk_mms(ps[:], w_ap, jbase, ch, CHUNK)
                if raw_off is None:
                    e_tile = epool.tile([B, CHUNK], F32)
                    nc.scalar.activation(
                        e_tile[:], ps[:],
                        mybir.ActivationFunctionType.Exp,
                        scale=1.0 / (MM_SCALE * MM_SCALE),
                        accum_out=sg_sb[:, ch:ch + 1],
                    )
                else:
                    nc.scalar.activation(
                        sg_sb[:, raw_off:raw_off + CHUNK], ps[:],
                        mybir.ActivationFunctionType.Exp,
                        scale=1.0 / (MM_SCALE * MM_SCALE),
                    )

            # ---- main stream: chunks 1..30 in groups, side path slotted in
            side_emitted = False
            for rep in range(nloop):
                ch0 = 1
                for gidx, nch in enumerate(PLAN):
                    with_wg = rep == 0 and gidx == SIDE_AT
                    if rep == 0 and gidx < 2:
                        w_tile = pre[gidx]
                    else:
                        w_tile = issue_group(nch, extra=WG_COLS if with_wg else 0)
                    if rep == 0 and gidx == 0:
                        # chunk 0 (from the c0 consts tile, after pack2)
                        chunk_compute(0, c0, W0_OFF)
                    if with_wg and not side_emitted:
                        wg_view[0] = w_tile[:, nch * KT * CHUNK:
                                            nch * KT * CHUNK + WG_COLS]
                        emit_side_path()
                        side_emitted = True
                    for j in range(nch):
                        chunk_compute(ch0 + j, w_tile, j * KT * CHUNK)
                    ch0 += nch
                assert side_emitted, "SIDE_AT must be within PLAN (and >= 2)"

                # ---- last RAWN chunks: pieces streamed last, raw-shipped.
                # The final piece streams as SUBLAST half-DMAs feeding one
                # psum tile, so its matmuls overlap the very last transfer.
                v0 = 0
                for pi, w in enumerate(TAIL):
                    sub = _piece_subs(pi, w)
                    nsub = w // sub
                    ps = pspool.tile([B, w], F32, tag="ps",
                                     padded_shape=[B, CHUNK])
                    for si in range(nsub):
                        pool = spool if nsub > 1 else lpool
                        p_tile = pool.tile([128, KT * sub], mm_dtype,
                                           padded_shape=[128, KT * (128 if nsub > 1 else CHUNK)],
                                           tag="p_tile" if nsub > 1 else "l_tile")
                        nc.sync.dma_start(
                            out=p_tile[:],
                            in_=wtc[:, src_pos[0]:src_pos[0] + KT * sub],
                        )
                        src_pos[0] += KT * sub
                        half = ps[:, si * sub:(si + 1) * sub]
                        boff = NS * CHUNK + v0 + si * sub
                        if use_bias:
                            nc.tensor.matmul(
                                half,
                                lhsT=ones_r,
                                rhs=bias_sb[:, boff:boff + sub],
                                start=True, stop=False,
                            )
                        if DR:
                            for k2 in range(KT // 2):
                                nc.tensor.matmul(
                                    half,
                                    lhsT=c0[:, P2_DR + k2 * 32:P2_DR + (k2 + 1) * 32]
                                        .rearrange("p (i m) -> p i m", i=2)[:, :, 0:B],
                                    rhs=p_tile[:, 2 * k2 * sub:(2 * k2 + 2) * sub]
                                        .rearrange("p (i n) -> p i n", i=2),
                                    start=(not use_bias and k2 == 0),
                                    stop=(k2 == KT // 2 - 1),
                                    perf_mode=mybir.MatmulPerfMode.DoubleRow,
                                )
                        else:
                            for k in range(KT):
                                nc.tensor.matmul(
                                    half,
                                    lhsT=ht_k(k),
                                    rhs=p_tile[:, k * sub:(k + 1) * sub],
                                    start=(not use_bias and k == 0),
                                    stop=(k == KT - 1),
                                )
                    nc.scalar.activation(
                        sg16_sb[:, SG_RAW + v0:SG_RAW + v0 + w], ps[:],
                        mybir.ActivationFunctionType.Exp,
                        scale=1.0 / (MM_SCALE * MM_SCALE),
                    )
                    v0 += w

            late = sum(TAIL[len(TAIL) - OSPLIT:]) if OSPLIT > 0 else 0
            split = SG16_COLS - late
            nc.sync.dma_start(out=sg_out[:], in_=sg_sb[:])
            nc.sync.dma_start(out=sg_out16[:, 0:split], in_=sg16_sb[:, 0:split])
            if late:
                nc.sync.dma_start(out=sg_out16[:, split:SG16_COLS],
                                  in_=sg16_sb[:, split:SG16_COLS])
    nc.compile()
    return nc


def _get_nc(nloop=1, use_bias=True):
    key = (str(MM_DTYPE), WBUFS, DR, PSBUFS, _PLAN, _TAIL, RAWN, SIDE_AT,
           OSPLIT, SUBLAST, SUBN, nloop, use_bias)
    if key not in _cached:
        _cached[key] = _build_nc(MM_DTYPE, nloop, use_bias)
    return _cached[key]


def _tile_k(x):
    # (D, N) -> (128, KT*N) with column blocks per contraction tile
    n = x.shape[1]
    return np.ascontiguousarray(
        x.reshape(KT, 128, n).transpose(1, 0, 2).reshape(128, KT * n)
    )


def _prep_core_inputs(W, bias_vec, points, ht):
    W4 = W.reshape(H, V, R, D)
    b3 = bias_vec.reshape(H, V, R)

    # gathered rows for p_eval: column order (h, r, b), sharded over cores
    rows = np.empty((NG,), np.int64)
    for h in range(H):
        for r in range(R):
            for b in range(B):
                rows[(h * R + r) * B + b] = h * V * R + int(points[b, h]) * R + r

    # pack2 content (fp8, scaled): DoubleRow-interleaved h^T
    pack2 = np.zeros((128, P2_COLS), np.float32)
    ht_t = _tile_k(ht.astype(np.float32)) * np.float32(MM_SCALE)  # (128, KT*B)
    for k2 in range(KT // 2):
        for i in range(2):
            k = 2 * k2 + i
            pack2[:, P2_DR + k2 * 32 + i * 16:P2_DR + k2 * 32 + i * 16 + B] = \
                ht_t[:, k * B:(k + 1) * B]
    pack2 = _to_mm(pack2)

    in_maps = []
    for c in range(NCORES):
        sl = slice(c * VSH, (c + 1) * VSH)
        # (h, v, r, k, p) -> (p, h, r, k, v): chunk-major per partition so
        # group DMAs are contiguous slices
        rows_c = rows[c * NGC:(c + 1) * NGC]
        wg = np.ascontiguousarray(W[rows_c, :].T)         # (D, NGC)
        wg_block = _to_mm(_tile_k(wg), MM_SCALE)

        s5 = W4[:, sl, :, :].reshape(H, VSH, R, KT, 128)
        slab = np.ascontiguousarray(s5.transpose(4, 0, 2, 3, 1))  # p,h,r,k,v
        slab = slab.reshape(128, NCHUNK, KT, VSH)
        # wg rides right after the SIDE_AT group's chunks
        nch_pre = 1 + sum(PLAN[:SIDE_AT + 1])
        main_a = slab[:, :nch_pre].reshape(128, nch_pre * KT * VSH)
        main_b = slab[:, nch_pre:NS].reshape(128, (NS - nch_pre) * KT * VSH)
        # tail: flatten the last RAWN chunks' vocab columns per k-tile so a
        # piece (v0, w) is one contiguous [k][v] block even across chunks
        tail_flat = slab[:, NS:].transpose(0, 2, 1, 3).reshape(128, KT, RAWN * VSH)
        piece_blocks = []
        v0 = 0
        for pi, w in enumerate(TAIL):
            sub = _piece_subs(pi, w)
            for si in range(w // sub):
                a = v0 + si * sub
                piece_blocks.append(
                    np.ascontiguousarray(tail_flat[:, :, a:a + sub])
                    .reshape(128, KT * sub))
            v0 += w
        wtc = np.concatenate(
            [pack2, _to_mm(main_a, MM_SCALE), wg_block, _to_mm(main_b, MM_SCALE)]
            + [_to_mm(pb, MM_SCALE) for pb in piece_blocks],
            axis=1)

        bc = np.ascontiguousarray(b3[:, sl, :].transpose(0, 2, 1)).reshape(-1)
        bcr = np.empty((1, NCHUNK * CHUNK + B), np.float32)
        bcr[0, :NCHUNK * CHUNK] = bc * np.float32(MM_SCALE * MM_SCALE)
        bcr[0, NCHUNK * CHUNK:] = 1.0
        bias_core = bcr.astype(mybir.dt.np(BF16))

        pack1 = np.zeros((1, P1_COLS), np.float32)
        pack1[0, P1_ONES:P1_ONES + B] = 1.0
        pack1[0, P1_BIASG:P1_BIASG + NGC] = \
            bias_vec[rows_c] * np.float32(MM_SCALE * MM_SCALE)

        in_maps.append({
            "pack1": pack1, "wtc": wtc, "bias_m": bias_core,
        })
    return in_maps


def _build_fast(nc):
    """Cache a jitted executor for this nc so repeat kernel() calls skip
    retracing/recompiling (mirrors bass2jax.run_bass_via_pjrt)."""
    import jax
    from concourse import bass2jax
    from concourse.bass2jax import _bass_exec_p, partition_id_tensor
    from jax.experimental.shard_map import shard_map
    from jax.sharding import Mesh, NamedSharding, PartitionSpec

    bass2jax.install_neuronx_cc_hook()
    partition_name = nc.partition_id_tensor.name if nc.partition_id_tensor else None
    in_names, out_names, out_avals, zero_outs = [], [], [], []
    for alloc in nc.m.functions[0].allocations:
        if not isinstance(alloc, mybir.MemoryLocationSet):
            continue
        name = alloc.memorylocations[0].name
        if alloc.kind == "ExternalInput":
            if name != partition_name:
                in_names.append(name)
        elif alloc.kind == "ExternalOutput":
            out_names.append(name)
            shape = tuple(alloc.tensor_shape)
            dtype = mybir.dt.np(alloc.dtype)
            out_avals.append(jax.core.ShapedArray(shape, dtype))
            zero_outs.append(np.zeros(shape, dtype))
    n_params = len(in_names)
    all_in = list(in_names) + list(out_names)
    if partition_name is not None:
        all_in.append(partition_name)

    def _body(*args):
        ops = list(args)
        if partition_name is not None:
            ops.append(partition_id_tensor())
        return tuple(
            _bass_exec_p.bind(
                *ops,
                out_avals=tuple(out_avals),
                in_names=tuple(all_in),
                out_names=tuple(out_names),
                lowering_input_output_aliases=(),
                sim_require_finite=True,
                sim_require_nnan=True,
                nc=nc,
            )
        )

    devices = jax.devices()[:NCORES]
    mesh = Mesh(np.asarray(devices), ("core",))
    spec = PartitionSpec("core")
    fn = jax.jit(
        shard_map(
            _body, mesh=mesh,
            in_specs=(spec,) * (n_params + len(out_names)),
            out_specs=(spec,) * len(out_names), check_rep=False,
        ),
        keep_unused=True,
    )
    _fast[id(nc)] = (fn, in_names, out_names, out_avals, zero_outs, mesh, spec)


def _run_cached(nc, in_maps):
    import jax

    fn, in_names, out_names, out_avals, zero_outs, mesh, spec = _fast[id(nc)]
    concat_in = [
        np.concatenate([np.asarray(in_maps[c][nm]) for c in range(NCORES)], axis=0)
        for nm in in_names
    ]
    concat_zero = [
        np.zeros((NCORES * z.shape[0], *z.shape[1:]), z.dtype) for z in zero_outs
    ]
    outs = fn(*concat_in, *concat_zero)
    return [
        {
            nm: np.asarray(outs[i]).reshape(NCORES, *out_avals[i].shape)[c]
            for i, nm in enumerate(out_names)
        }
        for c in range(NCORES)
    ]


def kernel(last_hidden_state, param_w, param_b, points):
    global _last_results
    from concourse.bass_utils import run_bass_kernel_spmd

    lhs = np.asarray(last_hidden_state, dtype=np.float32)
    W = np.ascontiguousarray(np.asarray(param_w, dtype=np.float32))
    bias_vec = np.asarray(param_b, dtype=np.float32)
    pts = np.asarray(points)

    ht = np.ascontiguousarray(lhs[:, -1, :].T)  # (D, B)
    in_maps = _prep_core_inputs(W, bias_vec, pts, ht)

    nc = _get_nc(use_bias=True)
    if id(nc) in _fast:
        results = _run_cached(nc, in_maps)
    else:
        res = run_bass_kernel_spmd(nc, in_maps, core_ids=list(range(NCORES)))
        _last_results = res
        results = res.results
        _build_fast(nc)

    s = np.zeros((B, NCHUNK), np.float64)
    g = np.zeros((B, NCHUNK), np.float64)
    for c, r in enumerate(results):
        sg = r["sg_out"].astype(np.float64)
        sg16 = r["sg_out16"].astype(np.float64)
        s[:, :NS] += sg[:, :NS]
        for i in range(RAWN):
            s[:, NS + i] += sg16[:, SG_RAW + i * CHUNK:
                                 SG_RAW + (i + 1) * CHUNK].sum(axis=1)
        eg = sg16[:, 0:NGC]
        for j in range(NHRC):
            for b in range(B):
                g[b, c * NHRC + j] = eg[b, j * B + b]
    s0, s1 = s[:, :R], s[:, R:]
    g0, g1 = g[:, :R], g[:, R:]
    norm_const = (s0 * s1).sum(axis=1)
    p_eval = (g0 * g1).sum(axis=1)
    return p_eval.astype(np.float32), norm_const.astype(np.float32)
